# revision 1
# baseline (speedup 1.0000x reference)
"""AttentionFusion kernel for 8 Trainium2 NeuronCores.

Reference computation (B=2, C=256, H=W=64, N=8192 tokens = 2 modalities x 4096):
    x    = concat(flat(feat0), flat(feat1))        # [B, N, C]
    Q,K,V = x @ W{q,k,v}.T + b{q,k,v}
    attn = softmax(Q @ K.T / 16)
    out  = (attn @ V) @ Wo.T + bo                  # [B, N, C]
    out  = mean over modalities -> [B, HW, C] -> [B, C, H, W]

Sharding: 8 cores = (2 batches) x (4 query groups). Core (b, g) computes
queries {g*1024..(g+1)*1024} of each modality (2048 rows) for batch b, with
full K/V (8192 tokens) computed locally. The modality mean pairs rows within
a core, so there is no cross-core communication at all.

Everything is computed in "transposed" (feature-on-partition) layout:
X^T comes for free from the channels-first input, and the final output
[C, HW] is exactly the transposed layout too, so no transposes are needed:
    K^T = Wk^T.T-matmuls over X^T          [256, 8192]
    Q^T = (Wq^T.T @ X^T + bq) / 16         [256, 2048]
    V   = X^T-tile-stationary matmuls      [8192, 256]
    S^T = (K^T-tile).T @ Q^T               [k, q] tiles, softmax axis = partitions
    P^T = exp(S^T)  (scores ~ N(0,1), no max subtraction needed)
    O^T = sum_k V-tile.T @ P^T-tile        accumulated in PSUM
    sums = ones.T @ (running DVE sum of P^T tiles), softmax normalize folded in
    out^T = Wo^T.T @ (O^T * 1/sums) * 0.5 + 0.5*bo_eff, summed over modalities

Numerics: float16 matmul inputs (1 cyc/row on the PE with separate cheap
LDWEIGHTS, ~5e-4 rel err for these O(1)-magnitude values), fp32 PSUM
accumulation, fp32 softmax statistics.
bk is dropped entirely (adds a per-query constant to scores -> softmax
invariant); bv is folded into bo_eff = bo + Wo @ bv on the host.
"""

import numpy as np

B, C, H, W = 2, 256, 64, 64
HW = H * W            # 4096
NTOK = 2 * HW         # 8192 tokens per batch (2 modalities)
NQ = 2048             # q columns per core
P = 128
KT = NTOK // P        # 64 k-tiles
QCH = 512             # q-chunk width
NCH = NQ // QCH       # 4 q-chunks per core
NCORES = 8

_compiled = {}


def _build():
    import concourse.bass as bass  # noqa: F401
    import concourse.mybir as mybir
    from concourse import bacc
    from concourse.tile import TileContext

    f32 = mybir.dt.float32
    f32r = mybir.dt.float16  # compute dtype (fp16: 1 cyc/row, separate LDW)
    COPY = mybir.ActivationFunctionType.Copy
    EXP = mybir.ActivationFunctionType.Exp

    nc = bacc.Bacc("TRN2", target_bir_lowering=False, debug=False,
                   num_devices=NCORES)

    xT = nc.dram_tensor("xT", [C, NTOK], f32, kind="ExternalInput")
    xTq = nc.dram_tensor("xTq", [C, NQ], f32, kind="ExternalInput")
    wqT_d = nc.dram_tensor("wqT", [C, C], f32, kind="ExternalInput")
    wkT_d = nc.dram_tensor("wkT", [C, C], f32, kind="ExternalInput")
    wvT_d = nc.dram_tensor("wvT", [C, C], f32, kind="ExternalInput")
    woT_d = nc.dram_tensor("woT", [C, C], f32, kind="ExternalInput")
    bq_d = nc.dram_tensor("bq_eff", [C], f32, kind="ExternalInput")
    bo_d = nc.dram_tensor("bo_eff", [C], f32, kind="ExternalInput")
    out_d = nc.dram_tensor("out", [C, NQ // 2], f32, kind="ExternalOutput")

    with TileContext(nc) as tc:
        with tc.tile_pool(name="const", bufs=1) as cpool, \
             tc.tile_pool(name="kTp", bufs=1) as kTp, \
             tc.tile_pool(name="qTp", bufs=1) as qTp, \
             tc.tile_pool(name="Vp", bufs=1) as Vp:

            # ---- constants: weights (cast to f32r), biases, ones ----
            def load_weight(dram, name):
                tiles = []
                for h in range(2):
                    raw = cpool.tile([P, C], f32, tag=f"{name}{h}raw")
                    nc.sync.dma_start(raw[:], dram.ap()[h * P:(h + 1) * P, :])
                    t = cpool.tile([P, C], f32r, tag=f"{name}{h}")
                    nc.scalar.activation(t[:], raw[:], COPY)
                    tiles.append(t)
                return tiles

            wqT = load_weight(wqT_d, "wq")
            wkT = load_weight(wkT_d, "wk")
            wvT = load_weight(wvT_d, "wv")
            woT = load_weight(woT_d, "wo")

            def load_bias(dram, name):
                tiles = []
                ap2 = dram.ap().rearrange("(p one) -> p one", one=1)
                for h in range(2):
                    t = cpool.tile([P, 1], f32, tag=f"{name}{h}")
                    nc.sync.dma_start(t[:], ap2[h * P:(h + 1) * P, :])
                    tiles.append(t)
                return tiles

            bq_sb = load_bias(bq_d, "bq")
            bo_sb = load_bias(bo_d, "bo")

            ones_col = cpool.tile([P, 1], f32, tag="ones_col")
            nc.vector.memset(ones_col[:], 1.0)
            allones = cpool.tile([P, P], f32, tag="allones")
            nc.vector.memset(allones[:], 1.0)

            # persistent activations
            kT = [kTp.tile([P, NTOK], f32r, tag=f"kT{h}", name=f"kT{h}")
                  for h in range(2)]
            qT = [qTp.tile([P, NQ], f32r, tag=f"qT{h}", name=f"qT{h}")
                  for h in range(2)]
            Vb = Vp.tile([P, KT * C], f32r, tag="Vb")  # [k-part, kt*256 + c]

            # ---- phase 1: stream X^T, compute K^T, V, Q^T ----
            with tc.tile_pool(name="xraw", bufs=3) as xrp, \
                 tc.tile_pool(name="xcast", bufs=3) as xcp, \
                 tc.tile_pool(name="p1ps", bufs=2, space="PSUM") as p1k, \
                 tc.tile_pool(name="p1vs", bufs=2, space="PSUM") as p1v:

                for j in range(NTOK // 512):
                    xc = []
                    for h in range(2):
                        raw = xrp.tile([P, 512], f32, tag=f"xr{h}")
                        nc.sync.dma_start(
                            raw[:], xT.ap()[h * P:(h + 1) * P,
                                            j * 512:(j + 1) * 512])
                        c = xcp.tile([P, 512], f32r, tag=f"xc{h}")
                        nc.scalar.activation(c[:], raw[:], COPY)
                        xc.append(c)
                    for ch in range(2):
                        kp = p1k.tile([P, 512], f32, tag="kp")
                        nc.tensor.matmul(kp[:], wkT[0][:, ch * P:(ch + 1) * P],
                                         xc[0][:], start=True, stop=False)
                        nc.tensor.matmul(kp[:], wkT[1][:, ch * P:(ch + 1) * P],
                                         xc[1][:], start=False, stop=True)
                        nc.scalar.activation(
                            kT[ch][:, j * 512:(j + 1) * 512], kp[:], COPY)
                    for t in range(4):
                        kt = 4 * j + t
                        vp = p1v.tile([P, C], f32, tag="vp")
                        nc.tensor.matmul(vp[:], xc[0][:, t * P:(t + 1) * P],
                                         wvT[0][:], start=True, stop=False)
                        nc.tensor.matmul(vp[:], xc[1][:, t * P:(t + 1) * P],
                                         wvT[1][:], start=False, stop=True)
                        nc.scalar.activation(
                            Vb[:, kt * C:(kt + 1) * C], vp[:], COPY)

                for j in range(NQ // 512):
                    xc = []
                    for h in range(2):
                        raw = xrp.tile([P, 512], f32, tag=f"xr{h}")
                        nc.sync.dma_start(
                            raw[:], xTq.ap()[h * P:(h + 1) * P,
                                             j * 512:(j + 1) * 512])
                        c = xcp.tile([P, 512], f32r, tag=f"xc{h}")
                        nc.scalar.activation(c[:], raw[:], COPY)
                        xc.append(c)
                    for ch in range(2):
                        qp = p1k.tile([P, 512], f32, tag="kp")
                        nc.tensor.matmul(qp[:], wqT[0][:, ch * P:(ch + 1) * P],
                                         xc[0][:], start=True, stop=False)
                        nc.tensor.matmul(qp[:], wqT[1][:, ch * P:(ch + 1) * P],
                                         xc[1][:], start=False, stop=True)
                        # Q^T/16 + bq/16 (host passes bq_eff = bq/16)
                        nc.vector.tensor_scalar(
                            qT[ch][:, j * 512:(j + 1) * 512], qp[:],
                            1.0 / 16.0, bq_sb[ch][:],
                            mybir.AluOpType.mult, mybir.AluOpType.add)

            # ---- phase 2: attention per q-chunk ----
            with tc.tile_pool(name="sps", bufs=3, space="PSUM") as sps, \
                 tc.tile_pool(name="ops", bufs=2, space="PSUM") as ops, \
                 tc.tile_pool(name="eps", bufs=3, space="PSUM") as eps, \
                 tc.tile_pool(name="pp", bufs=3) as pp, \
                 tc.tile_pool(name="sap", bufs=2) as sap, \
                 tc.tile_pool(name="nrm", bufs=2) as nrm, \
                 tc.tile_pool(name="ipd", bufs=2) as ipd, \
                 tc.tile_pool(name="bcp", bufs=2) as bcp, \
                 tc.tile_pool(name="tsp", bufs=6) as tsp, \
                 tc.tile_pool(name="osb", bufs=2) as osb:

                stash = {}
                # pairing order: modality-0 chunk immediately followed by its
                # modality-1 partner so stashed tiles are consumed right away
                for chunk in (0, 2, 1, 3):
                    qb = chunk * QCH
                    o_ps = [ops.tile([P, QCH], f32, tag="op", name=f"o{chunk}_{h}")
                            for h in range(2)]
                    sa = sap.tile([P, QCH], f32, tag="sa")
                    nc.vector.memset(sa[:], 0.0)

                    for kt in range(KT):
                        sp = sps.tile([P, QCH], f32, tag="sp")
                        nc.tensor.matmul(sp[:], kT[0][:, kt * P:(kt + 1) * P],
                                         qT[0][:, qb:qb + QCH],
                                         start=True, stop=False)
                        nc.tensor.matmul(sp[:], kT[1][:, kt * P:(kt + 1) * P],
                                         qT[1][:, qb:qb + QCH],
                                         start=False, stop=True)
                        p = pp.tile([P, QCH], f32r, tag="p")
                        nc.scalar.activation(p[:], sp[:], EXP)
                        first, last = kt == 0, kt == KT - 1
                        nc.tensor.matmul(o_ps[0][:],
                                         Vb[:, kt * C:kt * C + P],
                                         p[:], start=first, stop=last)
                        nc.tensor.matmul(o_ps[1][:],
                                         Vb[:, kt * C + P:(kt + 1) * C],
                                         p[:], start=first, stop=last)
                        nc.vector.tensor_add(sa[:], sa[:], p[:])

                    # softmax sums -> 1/sums broadcast to 128 partitions
                    sm = eps.tile([1, QCH], f32, tag="ep")
                    nc.tensor.matmul(sm[:], ones_col[:], sa[:],
                                     start=True, stop=True)
                    ip = ipd.tile([P, QCH], f32, tag="ip")
                    nc.vector.memset(ip[:], 0.0)
                    nc.vector.reciprocal(ip[0:1, :], sm[:])
                    bc_ps = eps.tile([P, QCH], f32, tag="ep")
                    nc.tensor.matmul(bc_ps[:], allones[:], ip[:],
                                     start=True, stop=True)
                    bc = bcp.tile([P, QCH], f32, tag="bc")
                    nc.scalar.activation(bc[:], bc_ps[:], COPY)

                    # normalize O^T, output projection, 0.5*(...)+0.5*bo_eff
                    n = []
                    for ch in range(2):
                        nt = nrm.tile([P, QCH], f32r, tag="no")
                        nc.vector.tensor_mul(nt[:], o_ps[ch][:], bc[:])
                        n.append(nt)
                    tts = []
                    for ch in range(2):
                        fp = eps.tile([P, QCH], f32, tag="ep")
                        nc.tensor.matmul(fp[:], woT[0][:, ch * P:(ch + 1) * P],
                                         n[0][:], start=True, stop=False)
                        nc.tensor.matmul(fp[:], woT[1][:, ch * P:(ch + 1) * P],
                                         n[1][:], start=False, stop=True)
                        tt = tsp.tile([P, QCH], f32, tag="ts")
                        # 0.5*f + 0.5*bo_eff (host passes bo_eff pre-halved)
                        nc.vector.tensor_scalar(
                            tt[:], fp[:], 0.5, bo_sb[ch][:],
                            mybir.AluOpType.mult, mybir.AluOpType.add)
                        tts.append(tt)

                    if chunk < NCH // 2:
                        stash[chunk] = tts
                    else:
                        prev = stash[chunk - NCH // 2]
                        ob = (chunk - NCH // 2) * QCH
                        for ch in range(2):
                            ot = osb.tile([P, QCH], f32, tag="os")
                            nc.vector.tensor_add(ot[:], tts[ch][:],
                                                 prev[ch][:])
                            nc.sync.dma_start(
                                out_d.ap()[ch * P:(ch + 1) * P,
                                           ob:ob + QCH], ot[:])

    nc.compile()
    return nc


def _get_compiled():
    if "nc" not in _compiled:
        _compiled["nc"] = _build()
    return _compiled["nc"]


def kernel(feat0, feat1, Wq, bq, Wk, bk, Wv, bv, Wo, bo):
    from concourse.bass_utils import run_bass_kernel_spmd

    feat0 = np.asarray(feat0, dtype=np.float32)
    feat1 = np.asarray(feat1, dtype=np.float32)
    Wq = np.asarray(Wq, dtype=np.float32)
    Wk = np.asarray(Wk, dtype=np.float32)
    Wv = np.asarray(Wv, dtype=np.float32)
    Wo = np.asarray(Wo, dtype=np.float32)
    bq = np.asarray(bq, dtype=np.float32)
    bv = np.asarray(bv, dtype=np.float32)
    bo = np.asarray(bo, dtype=np.float32)

    wqT = np.ascontiguousarray(Wq.T)
    wkT = np.ascontiguousarray(Wk.T)
    wvT = np.ascontiguousarray(Wv.T)
    woT = np.ascontiguousarray(Wo.T)
    bq_eff = np.ascontiguousarray(bq / 16.0)
    bo_eff = np.ascontiguousarray(0.5 * (bo + Wo @ bv))

    xT_all = [
        np.ascontiguousarray(
            np.concatenate([feat0[b].reshape(C, HW), feat1[b].reshape(C, HW)],
                           axis=1))
        for b in range(B)
    ]

    in_maps = []
    for core in range(NCORES):
        b, g = core // 4, core % 4
        cols0 = slice(g * 1024, (g + 1) * 1024)
        cols1 = slice(HW + g * 1024, HW + (g + 1) * 1024)
        xTq = np.ascontiguousarray(
            np.concatenate([xT_all[b][:, cols0], xT_all[b][:, cols1]],
                           axis=1))
        in_maps.append({
            "xT": xT_all[b], "xTq": xTq,
            "wqT": wqT, "wkT": wkT, "wvT": wvT, "woT": woT,
            "bq_eff": bq_eff, "bo_eff": bo_eff,
        })

    global _last_in_maps
    _last_in_maps = in_maps

    nc = _get_compiled()
    res = run_bass_kernel_spmd(nc, in_maps, core_ids=list(range(NCORES)))

    full = np.empty((B, C, HW), dtype=np.float32)
    for core in range(NCORES):
        b, g = core // 4, core % 4
        full[b][:, g * 1024:(g + 1) * 1024] = res.results[core]["out"]
    return full.reshape(B, C, H, W)



# revision 2
# speedup vs baseline: 1.1189x; 1.1189x over previous
"""AttentionFusion kernel for 8 Trainium2 NeuronCores (v2).

Reference computation (B=2, C=256, H=W=64, N=8192 tokens = 2 modalities x 4096):
    x    = concat(flat(feat0), flat(feat1))        # [B, N, C]
    Q,K,V = x @ W{q,k,v}.T + b{q,k,v}
    attn = softmax(Q @ K.T / 16)
    out  = (attn @ V) @ Wo.T + bo                  # [B, N, C]
    out  = mean over modalities -> [B, HW, C] -> [B, C, H, W]

Sharding: 8 cores = (2 batches) x (4 query groups). Core (b, g) computes
queries {g*1024..(g+1)*1024} of each modality (2048 rows) for batch b, with
full K/V (8192 tokens) computed locally. The modality mean pairs rows within
a core, so there is no cross-core communication at all.

Everything is computed in "transposed" (feature-on-partition) layout; no
transposes needed anywhere.

Host-side folds (all exact linear algebra, done in fp32/fp64):
  - X, weights pre-cast to fp16 on the host (no on-device casts).
  - W~v = Wo @ Wv: the output projection is folded into V, so
    out = softmax(S) @ V~ + bo_eff with V~ = X @ W~v.T and
    bo_eff = bo + Wo @ bv (the V bias rides through softmax rows
    summing to 1). The on-device output projection disappears.
  - bq_eff = bq / 16 (score scale folded), bk dropped (softmax-invariant).
  - Softmax normalization via exact power-of-two scale chain:
    sm = sums * 2^-10 (ones-column of value 2^-10), bc = 1/sm,
    out = (n0 + n1) * 2^-11 + bo_eff  =>  0.5 * (o/sums) summed over
    modalities, where 0.5 = modality mean.

Engine balance per core (target ~250us PE-bound):
  - PE: S^T = K^T.T @ Q^T and O^T = V.T @ P^T, 512-col matmuls, fp16.
  - ACT: exp() on [128,1024] 2-bank PSUM tiles (128 calls), K evacuation.
  - DVE: softmax-sum accumulation in fp16 (2x_1p mode), V evacuation,
    normalization, reciprocal_approx_fast.
"""

import numpy as np

B, C, H, W = 2, 256, 64, 64
HW = H * W            # 4096
NTOK = 2 * HW         # 8192 tokens per batch (2 modalities)
NQ = 2048             # q columns per core
P = 128
KT = NTOK // P        # 64 k-tiles
JT = NTOK // 1024     # 8 x-tiles of 1024 tokens
QCH = 1024            # q-chunk width (2 PSUM banks)
NCH = NQ // QCH       # 2 q-chunks per core (= modalities)
NCORES = 8

_compiled = {}


def _build():
    import concourse.bass as bass  # noqa: F401
    import concourse.mybir as mybir
    from concourse import bacc
    from concourse.tile import TileContext

    f32 = mybir.dt.float32
    f16 = mybir.dt.float16
    COPY = mybir.ActivationFunctionType.Copy
    EXP = mybir.ActivationFunctionType.Exp
    MULT = mybir.AluOpType.mult
    ADD = mybir.AluOpType.add

    nc = bacc.Bacc("TRN2", target_bir_lowering=False, debug=False,
                   num_devices=NCORES)

    xT = nc.dram_tensor("xT", [C, NTOK], f16, kind="ExternalInput")
    xTq = nc.dram_tensor("xTq", [C, NQ], f16, kind="ExternalInput")
    wqT_d = nc.dram_tensor("wqT", [C, C], f16, kind="ExternalInput")
    wkT_d = nc.dram_tensor("wkT", [C, C], f16, kind="ExternalInput")
    wvT_d = nc.dram_tensor("wvT", [C, C], f16, kind="ExternalInput")
    bq_d = nc.dram_tensor("bq_eff", [C], f32, kind="ExternalInput")
    bo_d = nc.dram_tensor("bo_eff", [C], f32, kind="ExternalInput")
    out_d = nc.dram_tensor("out", [C, QCH], f32, kind="ExternalOutput")

    with TileContext(nc) as tc:
        with tc.tile_pool(name="const", bufs=1) as cpool, \
             tc.tile_pool(name="kTp", bufs=1) as kTp, \
             tc.tile_pool(name="qTp", bufs=1) as qTp, \
             tc.tile_pool(name="Vp", bufs=1) as Vp:

            # ---- constants: fp16 weights (DMA direct), fp32 biases ----
            def load_weight(dram, name):
                tiles = []
                for h in range(2):
                    t = cpool.tile([P, C], f16, tag=f"{name}{h}")
                    nc.sync.dma_start(t[:], dram.ap()[h * P:(h + 1) * P, :])
                    tiles.append(t)
                return tiles

            wq_sb = load_weight(wqT_d, "wq")
            wk_sb = load_weight(wkT_d, "wk")
            wv_sb = load_weight(wvT_d, "wv")

            def load_bias(dram, name):
                tiles = []
                ap2 = dram.ap().rearrange("(p one) -> p one", one=1)
                for h in range(2):
                    t = cpool.tile([P, 1], f32, tag=f"{name}{h}")
                    nc.sync.dma_start(t[:], ap2[h * P:(h + 1) * P, :])
                    tiles.append(t)
                return tiles

            bq_sb = load_bias(bq_d, "bq")
            bo_sb = load_bias(bo_d, "bo")

            # ones-column scaled 2^-10: sums matmul gives 2*sums/2048
            vcol = cpool.tile([P, 1], f16, tag="vcol")
            nc.vector.memset(vcol[:], 1.0 / 1024.0)
            allones = cpool.tile([P, P], f16, tag="allones")
            nc.vector.memset(allones[:], 1.0)
            # persistent broadcast-source tiles: row 0 rewritten per chunk,
            # rows 1-127 stay zero forever
            ip = []
            for cidx in range(NCH):
                t = cpool.tile([P, QCH], f16, tag=f"ip{cidx}")
                nc.vector.memset(t[:], 0.0)
                ip.append(t)

            # persistent activations
            kT = [kTp.tile([P, NTOK], f16, tag=f"kT{h}", name=f"kT{h}")
                  for h in range(2)]
            qT = [qTp.tile([P, NQ], f16, tag=f"qT{h}", name=f"qT{h}")
                  for h in range(2)]
            Vb = Vp.tile([P, KT * C], f16, tag="Vb")  # [k-part, kt*256 + c]

            with tc.tile_pool(name="xcp", bufs=4) as xcp, \
                 tc.tile_pool(name="sps", bufs=2, space="PSUM") as sps, \
                 tc.tile_pool(name="ops", bufs=2, space="PSUM") as ops, \
                 tc.tile_pool(name="pp", bufs=3) as pp, \
                 tc.tile_pool(name="sap", bufs=2) as sap, \
                 tc.tile_pool(name="bcp", bufs=2) as bcp, \
                 tc.tile_pool(name="nnp", bufs=4) as nnp, \
                 tc.tile_pool(name="osb", bufs=4) as osb:

                # ---- phase Q: Q^T = (Wq^T.T @ Xq^T)/16 + bq/16 ----
                for mod in range(2):
                    xq = []
                    for h in range(2):
                        t = xcp.tile([P, 1024], f16, tag=f"xc{h}")
                        nc.sync.dma_start(
                            t[:], xTq.ap()[h * P:(h + 1) * P,
                                           mod * 1024:(mod + 1) * 1024])
                        xq.append(t)
                    for ch in range(2):
                        qp = sps.tile([P, 1024], f32, tag="sp")
                        for hc in range(2):
                            o = qp[:, hc * 512:(hc + 1) * 512]
                            nc.tensor.matmul(
                                o, wq_sb[0][:, ch * P:(ch + 1) * P],
                                xq[0][:, hc * 512:(hc + 1) * 512],
                                start=True, stop=False)
                            nc.tensor.matmul(
                                o, wq_sb[1][:, ch * P:(ch + 1) * P],
                                xq[1][:, hc * 512:(hc + 1) * 512],
                                start=False, stop=True)
                        nc.vector.tensor_scalar(
                            qT[ch][:, mod * 1024:(mod + 1) * 1024], qp[:],
                            1.0 / 16.0, bq_sb[ch][:], MULT, ADD)

                # ---- phase KV: stream X^T, compute K^T and V~ ----
                for j in range(JT):
                    xc = []
                    for h in range(2):
                        t = xcp.tile([P, 1024], f16, tag=f"xc{h}")
                        nc.sync.dma_start(
                            t[:], xT.ap()[h * P:(h + 1) * P,
                                          j * 1024:(j + 1) * 1024])
                        xc.append(t)
                    for ch in range(2):
                        kp = sps.tile([P, 1024], f32, tag="sp")
                        for hc in range(2):
                            o = kp[:, hc * 512:(hc + 1) * 512]
                            nc.tensor.matmul(
                                o, wk_sb[0][:, ch * P:(ch + 1) * P],
                                xc[0][:, hc * 512:(hc + 1) * 512],
                                start=True, stop=False)
                            nc.tensor.matmul(
                                o, wk_sb[1][:, ch * P:(ch + 1) * P],
                                xc[1][:, hc * 512:(hc + 1) * 512],
                                start=False, stop=True)
                        nc.scalar.activation(
                            kT[ch][:, j * 1024:(j + 1) * 1024], kp[:], COPY)
                    for g in range(2):
                        vp = ops.tile([P, 1024], f32, tag="op")
                        for t in range(4):
                            tok = g * 512 + t * P
                            o = vp[:, t * 256:(t + 1) * 256]
                            nc.tensor.matmul(
                                o, xc[0][:, tok:tok + P], wv_sb[0][:],
                                start=True, stop=False)
                            nc.tensor.matmul(
                                o, xc[1][:, tok:tok + P], wv_sb[1][:],
                                start=False, stop=True)
                        kt0 = j * 8 + g * 4
                        nc.vector.tensor_copy(
                            Vb[:, kt0 * C:(kt0 + 4) * C], vp[:])

                # ---- phase 2: attention per q-chunk (= modality) ----
                stash = None
                for chunk in range(NCH):
                    qb = chunk * QCH
                    o_ps = [ops.tile([P, QCH], f32, tag="op",
                                     name=f"o{chunk}_{ch}")
                            for ch in range(2)]
                    sa = sap.tile([P, QCH], f16, tag="sa")
                    nc.vector.memset(sa[:], 0.0)

                    for kt in range(KT):
                        sp = sps.tile([P, QCH], f32, tag="sp")
                        for hc in range(2):
                            o = sp[:, hc * 512:(hc + 1) * 512]
                            nc.tensor.matmul(
                                o, kT[0][:, kt * P:(kt + 1) * P],
                                qT[0][:, qb + hc * 512:qb + (hc + 1) * 512],
                                start=True, stop=False)
                            nc.tensor.matmul(
                                o, kT[1][:, kt * P:(kt + 1) * P],
                                qT[1][:, qb + hc * 512:qb + (hc + 1) * 512],
                                start=False, stop=True)
                        p = pp.tile([P, QCH], f16, tag="p")
                        nc.scalar.activation(p[:], sp[:], EXP)
                        first, last = kt == 0, kt == KT - 1
                        for ch in range(2):
                            for hc in range(2):
                                nc.tensor.matmul(
                                    o_ps[ch][:, hc * 512:(hc + 1) * 512],
                                    Vb[:, kt * C + ch * P:
                                          kt * C + (ch + 1) * P],
                                    p[:, hc * 512:(hc + 1) * 512],
                                    start=first, stop=last)
                        nc.vector.tensor_add(sa[:], sa[:], p[:])

                    # softmax sums (scaled 2^-10) -> broadcast -> 1/x
                    sm = sps.tile([1, QCH], f32, tag="sp")
                    for hc in range(2):
                        nc.tensor.matmul(
                            sm[:, hc * 512:(hc + 1) * 512], vcol[:],
                            sa[:, hc * 512:(hc + 1) * 512],
                            start=True, stop=True)
                    nc.vector.tensor_copy(ip[chunk][0:1, :], sm[:])
                    bc_ps = sps.tile([P, QCH], f32, tag="sp")
                    for hc in range(2):
                        nc.tensor.matmul(
                            bc_ps[:, hc * 512:(hc + 1) * 512], allones[:],
                            ip[chunk][:, hc * 512:(hc + 1) * 512],
                            start=True, stop=True)
                    bc = bcp.tile([P, QCH], f32, tag="bc")
                    nc.vector.reciprocal_approx_fast(bc[:], bc_ps[:])

                    nn_ = []
                    for ch in range(2):
                        t = nnp.tile([P, QCH], f32, tag="nn")
                        nc.vector.tensor_mul(t[:], o_ps[ch][:], bc[:])
                        nn_.append(t)

                    if chunk == 0:
                        stash = nn_
                    else:
                        for ch in range(2):
                            tmp = osb.tile([P, QCH], f32, tag="os")
                            nc.vector.tensor_add(tmp[:], nn_[ch][:],
                                                 stash[ch][:])
                            ot = osb.tile([P, QCH], f32, tag="os")
                            nc.vector.tensor_scalar(
                                ot[:], tmp[:], 1.0 / 2048.0, bo_sb[ch][:],
                                MULT, ADD)
                            nc.sync.dma_start(
                                out_d.ap()[ch * P:(ch + 1) * P, :], ot[:])

    nc.compile()
    return nc


def _get_compiled():
    if "nc" not in _compiled:
        _compiled["nc"] = _build()
    return _compiled["nc"]


def kernel(feat0, feat1, Wq, bq, Wk, bk, Wv, bv, Wo, bo):
    from concourse.bass_utils import run_bass_kernel_spmd

    feat0 = np.asarray(feat0, dtype=np.float32)
    feat1 = np.asarray(feat1, dtype=np.float32)
    Wq = np.asarray(Wq, dtype=np.float32)
    Wk = np.asarray(Wk, dtype=np.float32)
    Wv = np.asarray(Wv, dtype=np.float32)
    Wo = np.asarray(Wo, dtype=np.float32)
    bq = np.asarray(bq, dtype=np.float32)
    bv = np.asarray(bv, dtype=np.float32)
    bo = np.asarray(bo, dtype=np.float32)

    wqT = np.ascontiguousarray(Wq.T).astype(np.float16)
    wkT = np.ascontiguousarray(Wk.T).astype(np.float16)
    # fold output projection into V: V~ = X @ (Wo @ Wv).T
    wvT = np.ascontiguousarray((Wo @ Wv).T).astype(np.float16)
    bq_eff = np.ascontiguousarray(bq / 16.0)
    bo_eff = np.ascontiguousarray(bo + Wo @ bv)

    xT_all = [
        np.ascontiguousarray(
            np.concatenate([feat0[b].reshape(C, HW), feat1[b].reshape(C, HW)],
                           axis=1)).astype(np.float16)
        for b in range(B)
    ]

    in_maps = []
    for core in range(NCORES):
        b, g = core // 4, core % 4
        cols0 = slice(g * 1024, (g + 1) * 1024)
        cols1 = slice(HW + g * 1024, HW + (g + 1) * 1024)
        xTq = np.ascontiguousarray(
            np.concatenate([xT_all[b][:, cols0], xT_all[b][:, cols1]],
                           axis=1))
        in_maps.append({
            "xT": xT_all[b], "xTq": xTq,
            "wqT": wqT, "wkT": wkT, "wvT": wvT,
            "bq_eff": bq_eff, "bo_eff": bo_eff,
        })

    global _last_in_maps
    _last_in_maps = in_maps

    nc = _get_compiled()
    res = run_bass_kernel_spmd(nc, in_maps, core_ids=list(range(NCORES)))

    full = np.empty((B, C, HW), dtype=np.float32)
    for core in range(NCORES):
        b, g = core // 4, core % 4
        full[b][:, g * 1024:(g + 1) * 1024] = res.results[core]["out"]
    return full.reshape(B, C, H, W)
